# revision 14
# baseline (speedup 1.0000x reference)
"""Trainium2 Bass kernel for nn_ActorNetwork (2-layer LSTM [T=4,H=64] + 3-layer
MLP + log_softmax over a batch of 131072 13-dim states).

Strategy: pure data parallel over 8 NeuronCores (16384 samples/core).
Feature-major on-chip layout: hidden/gate units on SBUF partitions, samples on
the free axis. Samples are processed in "epochs" of 2 pairs = 2048 samples
(pair = two 512-sample subtiles A/B packed on partitions 0:64 / 64:128), so
every elementwise op runs [128, 1024] with all partitions busy and a 1024-wide
free dim that amortizes the per-instruction engine overhead.

Per LSTM step the four gates are computed as per-gate PSUM "waves":
one fp16 matmul per source (x-side / h-side) with a 1024-column moving
operand writing a 2-bank [128,1024] fp32 psum tile, ping-ponged across 2
wave tiles (4 banks total) so the PE streams while ACT drains. The other 4
psum banks hold the interleaved MLP (phase 2) pipeline, whose matmuls fill
PE idle slots and keep the HAM clock gate at full speed, helped by
LDWEIGHTS filler instructions. L0 biases ride ones-rows in the packed x;
L1 biases use the ACT bias operand. MLP biases ride ones-rows in ft/m1s/m2s.
Everything 16-bit is fp16 (DVE 2x perf mode + better mantissa than bf16);
psum and the cell-state chain stay in engine-native fp32/fp16 mix.
"""

import numpy as np

import concourse.bass as bass
import concourse.mybir as mybir
from concourse.tile import TileContext
from concourse.bass_utils import run_bass_kernel_spmd
from concourse.vector_clock import ScopedClock
import concourse.tile as _tile_mod

F16 = mybir.dt.float16
F32 = mybir.dt.float32
AF = mybir.ActivationFunctionType
ALU = mybir.AluOpType

P = 128
FD = 512            # samples per subtile
W = 1024            # free width of one epoch instruction (2 pairs)
H = 64
NCORES = 8
B_TOTAL = 131072
B_CORE = B_TOTAL // NCORES          # 16384
NEPOCH = B_CORE // (2 * W)          # 8 epochs x 2048 samples
NPAIR = B_CORE // (2 * FD)          # 16 pairs/core

# PyTorch gate order i, f, g, o -> block index in packed weights
GI, GF, GG, GO = 0, 1, 2, 3
GS = [slice(0, 64), slice(64, 128), slice(128, 192), slice(192, 256)]

LDW_FILL = 3  # ldweights fillers emitted after each gate wave (HAM keep-warm)

# ---------------------------------------------------------------------------
# walrus workaround: this toolchain rejects instructions carrying more than
# one sync wait; split excess waits onto same-engine nops inserted right
# before the offending instruction (identical engine-stream semantics).
_WAIT_LIMIT = 1


def _split_excess_waits(nc, limit=_WAIT_LIMIT):
    for f in nc.m.functions:
        for bb in f.blocks:
            snapshot = list(bb.instructions)
            out = []
            changed = False
            for inst in snapshot:
                si = getattr(inst, "sync_info", None)
                waits = list(si.on_wait) if si is not None else []
                if len(waits) > limit:
                    changed = True
                    extra, keep = waits[:-limit], waits[-limit:]
                    for w in extra:
                        b = nc.engines[inst.engine].nop(
                            nofuse=True, hint="wsplit"
                        )
                        ni = b.ins
                        cb = nc.cur_bb.bb
                        cb.instructions.remove(ni)
                        ni.sync_info = mybir.SyncInfo(
                            on_wait=[w], on_update=[]
                        )
                        out.append(ni)
                    inst.sync_info = mybir.SyncInfo(
                        on_wait=keep, on_update=list(si.on_update)
                    )
                out.append(inst)
            if changed:
                bb.instructions[:] = out


def _patched_drain_and_barrier(self, tick_clock, wait_clock):
    nc = self.nc
    drain_inst = nc.sync.drain()
    wait_clock.add_sem_waits(
        drain_inst.ins, ScopedClock({None: tick_clock.global_clock})
    )
    si = drain_inst.ins.sync_info
    waits = list(si.on_wait) if si is not None else []
    if len(waits) > _WAIT_LIMIT:
        drain_inst.ins.sync_info = mybir.SyncInfo(
            on_wait=waits[:_WAIT_LIMIT], on_update=list(si.on_update)
        )
        for k in range(_WAIT_LIMIT, len(waits), _WAIT_LIMIT):
            d2 = nc.sync.drain()
            d2.ins.sync_info = mybir.SyncInfo(
                on_wait=waits[k : k + _WAIT_LIMIT], on_update=[]
            )
    nc.all_engine_barrier()
    popped = nc._tile_sem_poison_stack.pop()
    assert popped is self._sem_poison
    nc.clear_and_free_semaphores(list(self.sems.allocated().values()))
    nc.all_engine_barrier()
    _split_excess_waits(nc)


_tile_mod.TileContext._drain_and_barrier = _patched_drain_and_barrier
# ---------------------------------------------------------------------------


class _Epoch:
    __slots__ = ("idx", "xp", "ft", "h0", "h1", "c0", "c1", "mlph")


def build_program(nepoch=NEPOCH):
    nc = bass.Bass("TRN2", num_devices=NCORES)
    ncols = nepoch * 2 * W

    xq = nc.declare_dram_parameter("xq", [4, 6, ncols // 2], F16, isOutput=False)
    fq = nc.declare_dram_parameter("fq", [6, ncols], F16, isOutput=False)
    wx6_d = nc.declare_dram_parameter("wx6", [6, 512], F16, isOutput=False)
    wh0_d = nc.declare_dram_parameter("wh0d", [128, 512], F16, isOutput=False)
    w1i_d = nc.declare_dram_parameter("w1i", [128, 512], F16, isOutput=False)
    w1r_d = nc.declare_dram_parameter("w1r", [128, 512], F16, isOutput=False)
    w1h_d = nc.declare_dram_parameter("w1h", [128, 30], F16, isOutput=False)
    w1f_d = nc.declare_dram_parameter("w1f", [38, 30], F16, isOutput=False)
    wm2_d = nc.declare_dram_parameter("wm2", [31, 10], F16, isOutput=False)
    wm3_d = nc.declare_dram_parameter("wm3", [43, 4], F16, isOutput=False)
    bl1_d = nc.declare_dram_parameter("bl1", [128, 4], F32, isOutput=False)
    o4_d = nc.declare_dram_parameter("ones4", [68, 1], F16, isOutput=False)
    no_d = nc.declare_dram_parameter("negones", [97, 4], F16, isOutput=False)
    out_d = nc.declare_dram_parameter("out", [4, ncols], F32, isOutput=True)
    warm_d = nc.declare_dram_parameter("warm", [1, 4], F32, isOutput=True)

    with TileContext(nc) as tc:
        with (
            tc.tile_pool(name="const", bufs=1) as const,
            tc.tile_pool(name="xp", bufs=2) as xp,
            tc.tile_pool(name="hp", bufs=2) as hp,
            tc.tile_pool(name="h1p", bufs=2) as h1p,
            tc.tile_pool(name="cs", bufs=4) as cs,
            tc.tile_pool(name="gp", bufs=3) as gp,
            tc.tile_pool(name="mp", bufs=3) as mp,
            tc.tile_pool(name="fp", bufs=3) as fp,
            tc.tile_pool(name="wv", bufs=2, space="PSUM") as wv,
            tc.tile_pool(name="pp2", bufs=1, space="PSUM") as pp2,
            tc.tile_pool(name="p2", bufs=2) as p2,
        ):
            # ---- constants -------------------------------------------------
            def cdma(name, dram, shape, dt=F16):
                t = const.tile(shape, dt, name=name)
                nc.sync.dma_start(t[:], dram[:, :])
                return t

            wx6 = cdma("wx6", wx6_d, [6, 512])
            wh0 = cdma("wh0", wh0_d, [128, 512])
            w1i = cdma("w1i", w1i_d, [128, 512])
            w1r = cdma("w1r", w1r_d, [128, 512])
            w1h = cdma("w1h", w1h_d, [128, 30])
            w1f = cdma("w1f", w1f_d, [38, 30])
            wm2 = cdma("wm2", wm2_d, [31, 10])
            wm3 = cdma("wm3", wm3_d, [43, 4])
            bl1 = cdma("bl1", bl1_d, [128, 4], F32)
            ones4 = cdma("ones4", o4_d, [68, 1])
            nones = cdma("nones", no_d, [97, 4])
            onesf = const.tile([1, 2048], F16, name="onesf")
            nc.vector.memset(onesf[:], 1.0)

            # ---- PE warm-up: dense matmul burst so the HAM clock gate
            # reaches 8/8 before the pipeline starts. Exported so it can't
            # be dead-code eliminated.
            wps = pp2.tile([128, 2048], F32, name="ps2")
            for k in range(28):
                q = k % 4
                nc.tensor.matmul(
                    wps[:, q * 512 : (q + 1) * 512],
                    lhsT=wh0[:, 0:128],
                    rhs=wh0[:, 0:512],
                    start=(k < 4),
                    stop=(k >= 24),
                    tile_position=(0, 0),
                )
            wsb = const.tile([1, 4], F32, name="wsb")
            nc.vector.tensor_copy(wsb[:], wps[0:1, 0:4])
            nc.sync.dma_start(warm_d[:, :], wsb[:])

            def open_epoch(e):
                ep = _Epoch()
                ep.idx = e
                ec = slice(e * W, (e + 1) * W)          # xq cols (pair-major)
                ep.xp = []
                for t in range(4):
                    x6 = xp.tile([6, W], F16, name=f"x6t{t}")
                    nc.sync.dma_start(x6[:], xq[t, :, ec])
                    ep.xp.append(x6)
                ep.ft = []
                for i in range(2):
                    pg = 2 * e + i                       # global pair index
                    ca = slice(pg * 2 * FD, pg * 2 * FD + FD)
                    cb = slice(pg * 2 * FD + FD, (pg + 1) * 2 * FD)
                    ft = fp.tile([38, FD], F16, name=f"ft{i}")
                    nc.sync.dma_start(ft[0:6, :], fq[:, ca])
                    nc.sync.dma_start(ft[32:38, :], fq[:, cb])
                    ep.ft.append(ft)
                ep.h0 = [None] * 4
                ep.h1 = [None] * 4
                ep.c0 = [None] * 4
                ep.c1 = [None] * 4
                ep.mlph = mp.tile([P, W], F16, name="mlph")
                return ep

            def emit_step(ep, s):
                layer, t = divmod(s, 4)
                if layer == 0:
                    wsrc, wrec, rec = wx6, wh0, ep.h0
                    xtile = ep.xp[t]
                else:
                    wsrc, wrec, rec = w1i, w1r, ep.h1
                    xtile = ep.h0[t]

                gates = [GG, GI, GO] if t == 0 else [GG, GI, GF, GO]
                ps = {}
                for g in gates:
                    pt = wv.tile([P, W], F32, name=f"wv{g}", tag="wv")
                    for hw in (0, 1):
                        hc = slice(hw * FD, (hw + 1) * FD)
                        nc.tensor.matmul(
                            pt[:, hc],
                            lhsT=wsrc[:, 128 * g : 128 * (g + 1)],
                            rhs=xtile[:, hc],
                            start=True,
                            stop=(t == 0),
                            tile_position=(0, 0),
                        )
                        if t > 0:
                            nc.tensor.matmul(
                                pt[:, hc],
                                lhsT=wrec[:, 128 * g : 128 * (g + 1)],
                                rhs=rec[t - 1][:, hc],
                                start=False,
                                stop=True,
                                tile_position=(0, 0),
                            )
                    for _ in range(LDW_FILL):
                        nc.tensor.ldweights(weights=wh0[:, 0:128])
                    # activation straight after each wave so psum drains fast
                    sg = gp.tile([P, W], F16, name=f"sg{g}")
                    if layer == 1:
                        nc.scalar.activation(
                            sg[:], pt[:, :],
                            AF.Tanh if g == GG else AF.Sigmoid,
                            bias=bl1[:, g : g + 1],
                        )
                    else:
                        nc.scalar.activation(
                            sg[:], pt[:, :],
                            AF.Tanh if g == GG else AF.Sigmoid,
                        )
                    ps[g] = sg

                cst = ep.c0 if layer == 0 else ep.c1
                if t == 0:
                    cn = cs.tile([P, W], F16, name="c")
                    nc.vector.tensor_mul(cn[:], ps[GG][:], ps[GI][:])
                else:
                    t1 = gp.tile([P, W], F16, name="t1")
                    nc.vector.tensor_mul(t1[:], ps[GG][:], ps[GI][:])
                    t2 = gp.tile([P, W], F16, name="t2")
                    nc.vector.tensor_mul(t2[:], ps[GF][:], cst[t - 1][:])
                    cn = cs.tile([P, W], F16, name="c")
                    nc.vector.tensor_add(cn[:], t1[:], t2[:])
                cst[t] = cn
                tcx = gp.tile([P, W], F16, name="tcx")
                nc.scalar.activation(tcx[:], cst[t][:], AF.Tanh)

                if layer == 0:
                    dst = hp.tile([P, W], F16, name=f"h0t{t}")
                    ep.h0[t] = dst
                elif t < 3:
                    dst = h1p.tile([P, W], F16, name=f"h1t{t}")
                    ep.h1[t] = dst
                else:
                    dst = ep.mlph
                nc.vector.tensor_mul(dst[:], ps[GO][:], tcx[:])

            # ---- phase 2: MLP + log_softmax for one finished epoch,
            # emitted in dependency-ordered stages so it can interleave
            # with later epochs' LSTM steps.
            def phase2_stages(ep):
                e = ep.idx
                ps = pp2.tile([128, 2048], F32, name="ps2")

                def s_mlp1():
                    # mlph [128, 1024]: pair i cols [512i, 512i+512)... the
                    # epoch free axis is pair-major: cols 0:512 pair0, 512:1024
                    # pair1; rows 0:64 subtile A, 64:128 subtile B.
                    for i in range(2):
                        pcols = slice(i * FD, (i + 1) * FD)
                        for half in (0, 1):
                            c = slice(1024 * i + 512 * half,
                                      1024 * i + 512 * half + 512)
                            rows = slice(0, 64) if half == 0 else slice(64, 128)
                            frows = slice(0, 6) if half == 0 else slice(32, 38)
                            nc.tensor.matmul(
                                ps[0:30, c], lhsT=w1h[rows, :],
                                rhs=ep.mlph[rows, pcols],
                                start=True, stop=False,
                                tile_position=(0 if half == 0 else 64, 0),
                            )
                            nc.tensor.matmul(
                                ps[0:30, c], lhsT=w1f[frows, :],
                                rhs=ep.ft[i][frows, :],
                                start=False, stop=True,
                                tile_position=(0 if half == 0 else 32, 0),
                            )

                m1s = p2.tile([31, 2048], F16, name="m1s", tag="m1s")
                m2s = p2.tile([43, 2048], F16, name="m2s", tag="m2s")
                es = p2.tile([68, 2048], F16, name="es", tag="es")
                ls = p2.tile([97, 2048], F16, name="ls", tag="ls")
                fo = p2.tile([68, 2048], F32, name="fo", tag="fo", bufs=1)

                def s_relu1():
                    nc.sync.dma_start(m1s[30:31, :], onesf[:, :])
                    nc.vector.tensor_scalar_max(m1s[0:30, :], ps[0:30, :], 0.0)

                def s_mlp2():
                    for j in range(4):
                        c = slice(512 * j, 512 * (j + 1))
                        nc.tensor.matmul(
                            ps[32:42, c], lhsT=wm2[:, :], rhs=m1s[:, c],
                            start=True, stop=True, tile_position=(0, 32),
                        )

                def s_relu2():
                    nc.sync.dma_start(m2s[42:43, :], onesf[:, :])
                    nc.vector.tensor_scalar_max(
                        m2s[32:42, :], ps[32:42, :], 0.0
                    )

                def s_mlp3():
                    for j in range(4):
                        c = slice(512 * j, 512 * (j + 1))
                        nc.tensor.matmul(
                            ps[64:68, c], lhsT=wm3[32:43, :],
                            rhs=m2s[32:43, c],
                            start=True, stop=True, tile_position=(32, 64),
                        )

                def s_exp():
                    nc.scalar.activation(es[64:68, :], ps[64:68, :], AF.Exp)
                    for j in range(4):
                        c = slice(512 * j, 512 * (j + 1))
                        nc.tensor.matmul(
                            ps[96:97, c], lhsT=ones4[64:68, :],
                            rhs=es[64:68, c],
                            start=True, stop=True, tile_position=(64, 96),
                        )

                def s_ln():
                    nc.scalar.activation(ls[96:97, :], ps[96:97, :], AF.Ln)
                    for j in range(4):
                        c = slice(512 * j, 512 * (j + 1))
                        nc.tensor.matmul(
                            ps[64:68, c], lhsT=nones[96:97, :],
                            rhs=ls[96:97, c],
                            start=False, stop=True, tile_position=(96, 64),
                            skip_group_check=True,
                        )

                def s_out():
                    nc.vector.tensor_copy(fo[64:68, :], ps[64:68, :])
                    nc.sync.dma_start(
                        out_d[:, 2048 * e : 2048 * (e + 1)], fo[64:68, :]
                    )

                return [s_mlp1, s_relu1, s_mlp2, s_relu2, s_mlp3, s_exp,
                        s_ln, s_out]

            # ==== main schedule: two epochs in flight, phase-2 of the two
            # previous epochs interleaved into the 8 step-slots.
            pending = []        # phase-2 stage lists awaiting emission
            for e0 in range(0, nepoch, 2):
                epA = open_epoch(e0)
                epB = open_epoch(e0 + 1)
                # stage schedule: 8 slots, two pipelines of 8 stages each:
                # previous window's epochs (if any)
                for s in range(8):
                    emit_step(epA, s)
                    emit_step(epB, s)
                    if pending:
                        if s < 4 and len(pending) >= 1:
                            for st in pending[0][2 * s : 2 * s + 2]:
                                st()
                        elif s >= 4 and len(pending) >= 2:
                            for st in pending[1][2 * (s - 4) : 2 * (s - 4) + 2]:
                                st()
                if pending:
                    pending = []
                pending = [phase2_stages(epA), phase2_stages(epB)]
            # drain the last two epochs' MLPs
            for stages in pending:
                for st in stages:
                    st()

    return nc


def pack_weights(Wih0, Whh0, bih0, bhh0, Wih1, Whh1, bih1, bhh1,
                 W1, b1, W2, b2, W3, b3):
    f16 = np.float16
    b0 = bih0 + bhh0
    b1l = bih1 + bhh1
    wx6 = np.zeros((6, 512), np.float32)
    wh0 = np.zeros((128, 512), np.float32)
    w1i = np.zeros((128, 512), np.float32)
    w1r = np.zeros((128, 512), np.float32)
    bl1 = np.zeros((128, 4), np.float32)
    for g in range(4):
        sl = GS[g]
        cA = slice(128 * g, 128 * g + 64)
        cB = slice(128 * g + 64, 128 * g + 128)
        wx6[0:2, cA] = Wih0[sl].T
        wx6[2, cA] = b0[sl]
        wx6[3:5, cB] = Wih0[sl].T
        wx6[5, cB] = b0[sl]
        wh0[0:64, cA] = Whh0[sl].T
        wh0[64:128, cB] = Whh0[sl].T
        w1i[0:64, cA] = Wih1[sl].T
        w1i[64:128, cB] = Wih1[sl].T
        w1r[0:64, cA] = Whh1[sl].T
        w1r[64:128, cB] = Whh1[sl].T
        bl1[0:64, g] = b1l[sl]
        bl1[64:128, g] = b1l[sl]
    w1h = np.zeros((128, 30), np.float32)
    w1h[0:64] = W1[:, 0:64].T
    w1h[64:128] = W1[:, 0:64].T
    w1f = np.zeros((38, 30), np.float32)
    w1f[0:5] = W1[:, 64:69].T
    w1f[5] = b1
    w1f[32:37] = W1[:, 64:69].T
    w1f[37] = b1
    wm2 = np.zeros((31, 10), np.float32)
    wm2[0:30] = W2.T
    wm2[30] = b2
    wm3 = np.zeros((43, 4), np.float32)
    wm3[32:42] = W3.T
    wm3[42] = b3
    ones4 = np.zeros((68, 1), np.float32)
    ones4[64:68] = 1.0
    negones = np.zeros((97, 4), np.float32)
    negones[96] = -1.0
    return {
        "wx6": wx6.astype(f16),
        "wh0d": wh0.astype(f16),
        "w1i": w1i.astype(f16),
        "w1r": w1r.astype(f16),
        "w1h": w1h.astype(f16),
        "w1f": w1f.astype(f16),
        "wm2": wm2.astype(f16),
        "wm3": wm3.astype(f16),
        "bl1": bl1,
        "ones4": ones4.astype(f16),
        "negones": negones.astype(f16),
    }


def pack_x(xs):
    """xs: [n, 13] f32 -> (xq [4, 6, n//2], fq [6, n]) f16.

    Pair p covers samples [1024p, 1024p+1024): subtile A = first 512 (on
    partition rows 0:2 of xq / 0:5 of fq-slice), B = last 512 (rows 3:5).
    """
    n = xs.shape[0]
    npair = n // (2 * FD)
    a = xs.reshape(npair, 2, FD, 13)
    A = a[:, 0].reshape(npair * FD, 13)
    Bv = a[:, 1].reshape(npair * FD, 13)
    xqv = np.zeros((4, 6, npair * FD), np.float32)
    for t in range(4):
        xqv[t, 0:2] = A[:, 2 * t : 2 * t + 2].T
        xqv[t, 2] = 1.0
        xqv[t, 3:5] = Bv[:, 2 * t : 2 * t + 2].T
        xqv[t, 5] = 1.0
    fqv = np.ones((6, n), np.float32)
    fqv[0:5] = xs[:, 8:13].T
    return xqv.astype(np.float16), fqv.astype(np.float16)


_cached = {}


def run_cores(x, weights, trace=False):
    """x: [B_TOTAL, 13] f32. Returns (out [B_TOTAL, 4] f32, results)."""
    key = "prog"
    if key not in _cached:
        _cached[key] = build_program(NEPOCH)
    nc = _cached[key]
    in_maps = []
    for c in range(NCORES):
        xs = x[c * B_CORE : (c + 1) * B_CORE]
        m = dict(weights)
        m["xq"], m["fq"] = pack_x(xs)
        in_maps.append(m)
    res = run_bass_kernel_spmd(
        nc, in_maps, core_ids=list(range(NCORES)), trace=trace
    )
    outs = [res.results[c]["out"] for c in range(NCORES)]  # [4, 16384] each
    full = np.concatenate([o.T for o in outs], axis=0)     # [B_TOTAL, 4]
    return np.ascontiguousarray(full, dtype=np.float32), res


def kernel(x, Wih0, Whh0, bih0, bhh0, Wih1, Whh1, bih1, bhh1,
           W1, b1, W2, b2, W3, b3):
    args = [np.asarray(a, dtype=np.float32) for a in (
        Wih0, Whh0, bih0, bhh0, Wih1, Whh1, bih1, bhh1, W1, b1, W2, b2, W3, b3
    )]
    weights = pack_weights(*args)
    out, _ = run_cores(np.asarray(x, dtype=np.float32), weights)
    return out
